# revision 14
# baseline (speedup 1.0000x reference)
"""DCT-feature-extractor kernel for 8 Trainium2 NeuronCores.

Math collapse: the reference keeps only dct[0, 0:4] of each 8x8 block's 2-D
orthonormal-DFT real part.  Row 0 of the DFT matrix is constant (Fr[0,:] =
1/sqrt(8), Fi[0,:] = 0), so

    feat[m] = sum_l G[m, l] * colsum[l],   G[m, l] = cos(2*pi*m*l/8) / 8,

where colsum[l] is the column sum of the 8x8 block.  The whole module is then

    out[b, o] = sum_{i,j,m} W[o, (i*64+j)*4+m] * feat[b,i,j,m] + bias[o].

Sharding: split the 512 image rows (block-row groups i) and the matching
weight columns across 8 cores -> each core reads 4 MB of image + 4 MB of
weight (the 64 MB of essential HBM traffic / 8, no replication) and emits a
[32, 512] partial product; the host sums partials and adds the bias.

Per-core schedule (HWDGE rings are FIFO per issuing engine, so program order
on nc.sync is the transfer order):
  SP ring: x in 4 x 1MB chunks (host pre-shuffled so each chunk is 8KB-run
           contiguous), then the weight in 8 x 0.5MB chunks so the final
           matmuls can chase arriving chunks.
  ACT ring: constants in, [32, 512] partial out.
  DVE: column-sum tree adds per x chunk.
  PE:  transpose y -> yT, block-diag-G matmul -> featsT, 16 accumulating
       matmuls vs the reordered W^T shard.
  ACT: all PSUM -> SBUF copies (keeps DVE free for the adds).
"""

import numpy as np

import concourse.bacc as bacc
import concourse.mybir as mybir
from concourse.bass_utils import run_bass_kernel_spmd
from concourse.tile import TileContext

N_CORES = 8
B = 32            # batch
H = 512           # image height
WD = 512          # image width
BS = 8            # dct block size
NF = 4            # kept dct coefficients per block
OUT = 512         # linear output dim
RPC = H // N_CORES          # 64 rows per core
IPC = RPC // BS             # 8 block-rows per core
F32 = mybir.dt.float32
F32R = mybir.dt.float32r   # full-rate fp32 PE path (moving dim >= 256)

N_WT_CHUNKS = 8   # weight streamed in 8 chunks of 2 output-tiles each


def _g_mat():
    m = np.arange(NF)[:, None].astype(np.float64)
    l = np.arange(BS)[None, :].astype(np.float64)
    return (np.cos(2.0 * np.pi * m * l / BS) / 8.0).astype(np.float32)  # [4, 8]


def _consts():
    """[128, 384] = identity | G_lo | G_hi.

    G_*[p=(j16,l8), q=(wc2,j16',m4)] = G[m, l] * (j16 == j16'), 'lo' filling
    q < 64 and 'hi' q >= 64, so two accumulating matmuls (rhs = yT of w-chunk
    2*fc, 2*fc+1) yield a [128, 256] featsT tile without partition offsets.
    """
    g = _g_mat()
    block = np.zeros((128, 64), np.float32)
    for j in range(16):
        block[j * 8:(j + 1) * 8, j * 4:(j + 1) * 4] = g.T  # [l, m]
    c = np.zeros((128, 384), np.float32)
    c[:, :128] = np.eye(128, dtype=np.float32)
    c[:, 128:192] = block   # lo: columns 0..63 of G_lo
    c[:, 320:384] = block   # hi: columns 64..127 of G_hi
    return c


def _build_bass():
    nc = bacc.Bacc("TRN2", target_bir_lowering=False, debug=False)
    # x host-prepped: [h, p=(b16, i8), f=(wh2, a8, w256)]
    x = nc.dram_tensor("x", [2, 128, 2 * BS * 256], F32, kind="ExternalInput")
    # wt host-prepped: [p=(wc2,j16,m), t'=(fc,i), o]
    wt = nc.dram_tensor("wt", [128, 2 * IPC, OUT], F32, kind="ExternalInput")
    out = nc.dram_tensor("out", [4 * B, OUT], F32, kind="ExternalOutput")
    cst_h = nc.inline_tensor(_consts(), "cst")

    with TileContext(nc) as tc:
        with (
            tc.tile_pool(name="sb", bufs=1) as sb,
            tc.tile_pool(name="ps", bufs=1, space="PSUM") as ps,
        ):
            # ---- DMA program order == HWDGE FIFO order ----
            # SP ring: x chunks (wh-major so w-half 0 completes first), then wt
            xt = [
                sb.tile([128, 2 * BS * 256], F32, tag=f"x{h}", name=f"x{h}")
                for h in range(2)
            ]
            for wh in range(2):
                for h in range(2):
                    nc.sync.dma_start(
                        out=xt[h][:, wh * 2048:(wh + 1) * 2048],
                        in_=x.ap()[h][:, wh * 2048:(wh + 1) * 2048],
                    )
            wts = sb.tile([128, 2 * IPC * OUT], F32, tag="wt")
            wr = wt.ap().rearrange("p t o -> p (t o)")
            wck = 2 * IPC * OUT // N_WT_CHUNKS
            for k in range(N_WT_CHUNKS):
                nc.sync.dma_start(
                    out=wts[:, k * wck:(k + 1) * wck],
                    in_=wr[:, k * wck:(k + 1) * wck],
                )
            # ACT ring: constants (identity + G halves)
            cst = sb.tile([128, 384], F32, tag="cst")
            nc.scalar.dma_start(out=cst[:, :], in_=cst_h.ap())
            ident, glo, ghi = cst[:, 0:128], cst[:, 128:256], cst[:, 256:384]

            # ---- stage 1: column sums (DVE), per (h, w-half) chunk ----
            ys = [sb.tile([128, WD], F32, tag=f"y{h}", name=f"y{h}") for h in range(2)]
            for wh in range(2):
                for h in range(2):
                    t, base = xt[h], wh * 2048
                    nc.vector.tensor_add(
                        t[:, base:base + 1024],
                        t[:, base:base + 1024],
                        t[:, base + 1024:base + 2048],
                    )
                    nc.vector.tensor_add(
                        t[:, base:base + 512],
                        t[:, base:base + 512],
                        t[:, base + 512:base + 1024],
                    )
                    nc.vector.tensor_add(
                        ys[h][:, wh * 256:(wh + 1) * 256],
                        t[:, base:base + 256],
                        t[:, base + 256:base + 512],
                    )

            # ---- stage 1.5 + 2 per w-half: transpose then block-diag G ----
            fts = []
            for fc in range(2):          # fc == wh
                yts = []
                for wc2 in range(2):
                    wc = 2 * fc + wc2
                    pyt = ps.tile([128, 256], F32, tag=f"pyt{wc}")
                    for h in range(2):
                        nc.tensor.transpose(
                            pyt[:, h * 128:(h + 1) * 128],
                            ys[h][:, wc * 128:(wc + 1) * 128],
                            ident,
                        )
                    yt = sb.tile([128, 256], F32, tag=f"yt{wc}")
                    nc.vector.tensor_copy(yt[:, :], pyt[:, :])
                    yts.append(yt)
                pft = ps.tile([128, 256], F32, tag=f"pft{fc}")
                nc.tensor.matmul(pft[:, :], glo, yts[0][:, :], start=True, stop=False)
                nc.tensor.matmul(pft[:, :], ghi, yts[1][:, :], start=False, stop=True)
                ft = sb.tile([128, 256], F32, tag=f"ft{fc}")
                nc.vector.tensor_copy(ft[:, :], pft[:, :])
                fts.append(ft)

            # ---- stage 3: 16 accumulating matmuls spread over the 4 PE
            # column groups (out partition offset 32*g -> tile_position), so
            # weight loads of one group overlap matmuls of another ----
            pout = ps.tile([128, OUT], F32, tag="pout")
            for fc in range(2):
                for i in range(IPC):
                    t = fc * IPC + i
                    g = t % 4
                    nc.tensor.matmul(
                        pout[32 * g:32 * (g + 1), :],
                        fts[fc][:, i::IPC],
                        wts[:, t * OUT:(t + 1) * OUT],
                        start=(t < 4),
                        stop=(t >= 2 * IPC - 4),
                        tile_position=(0, 32 * g),
                        skip_group_check=True,
                    )
            # ship all 4 col-group partials; host sums the groups
            outs = sb.tile([128, OUT], F32, tag="outs")
            nc.vector.tensor_copy(outs[:, :], pout[:, :])
            nc.scalar.dma_start(out=out.ap(), in_=outs[:, :])

    nc.compile()
    return nc


_NC_CACHE = None


def _get_nc():
    global _NC_CACHE
    if _NC_CACHE is None:
        _NC_CACHE = _build_bass()
    return _NC_CACHE


def make_in_maps(imgs, weight):
    """Per-core input dicts: shuffled channel-0 row slice + weight shard."""
    wr = weight.reshape(OUT, H // BS, WD // BS, NF)  # [o, i_glob, j, m]
    in_maps = []
    for c in range(N_CORES):
        xc = imgs[:, 0, RPC * c:RPC * (c + 1), :]    # [32, 64, 512]
        # -> [h, (b16, i8), (wh2, a8, w256)]
        xd = xc.reshape(2, 16, IPC, BS, 2, 256).transpose(0, 1, 2, 4, 3, 5)
        xd = np.ascontiguousarray(xd.reshape(2, 128, 2 * BS * 256))
        wc = wr[:, IPC * c:IPC * (c + 1)]            # [o, i, j, m]
        # p = wc2*64 + j16*4 + m (j = fc*32 + wc2*16 + j16), t' = fc*8 + i
        wtc = wc.reshape(OUT, IPC, 2, 2, 16, NF)     # o, i, fc, wc2, j16, m
        wtc = wtc.transpose(3, 4, 5, 2, 1, 0)        # wc2, j16, m, fc, i, o
        wtc = np.ascontiguousarray(wtc.reshape(128, 2 * IPC, OUT))
        in_maps.append({"x": xd, "wt": wtc})
    return in_maps


def kernel(imgs_tensors, weight, bias, block_size=8, num_features=4, **_):
    assert int(block_size) == BS and int(num_features) == NF
    imgs = np.ascontiguousarray(np.asarray(imgs_tensors, dtype=np.float32))
    w = np.ascontiguousarray(np.asarray(weight, dtype=np.float32))
    b = np.asarray(bias, dtype=np.float32)
    assert imgs.shape == (B, 3, H, WD) and w.shape == (OUT, H // BS * WD // BS * NF)

    nc = _get_nc()
    res = run_bass_kernel_spmd(nc, make_in_maps(imgs, w), core_ids=list(range(N_CORES)))
    acc = np.zeros((B, OUT), np.float32)
    for r in res.results:
        acc += r["out"].reshape(4, B, OUT).sum(axis=0)
    return (acc + b[None, :]).astype(np.float32)


# revision 15
# speedup vs baseline: 1.0012x; 1.0012x over previous
"""DCT-feature-extractor kernel for 8 Trainium2 NeuronCores.

Math collapse: the reference keeps only dct[0, 0:4] of each 8x8 block's 2-D
orthonormal-DFT real part.  Row 0 of the DFT matrix is constant (Fr[0,:] =
1/sqrt(8), Fi[0,:] = 0), so

    feat[m] = sum_l G[m, l] * colsum[l],   G[m, l] = cos(2*pi*m*l/8) / 8,

where colsum[l] is the column sum of the 8x8 block.  The whole module is then

    out[b, o] = sum_{i,j,m} W[o, (i*64+j)*4+m] * feat[b,i,j,m] + bias[o].

Sharding: split the 512 image rows (block-row groups i) and the matching
weight columns across 8 cores -> each core reads 4 MB of image + 4 MB of
weight (the 64 MB of essential HBM traffic / 8, no replication) and emits a
[32, 512] partial product; the host sums partials and adds the bias.

Per-core schedule (HWDGE rings are FIFO per issuing engine, so program order
on nc.sync is the transfer order):
  SP ring: x in 4 x 1MB chunks (host pre-shuffled so each chunk is 8KB-run
           contiguous), then the weight in 8 x 0.5MB chunks so the final
           matmuls can chase arriving chunks.
  ACT ring: constants in, [32, 512] partial out.
  DVE: column-sum tree adds per x chunk.
  PE:  transpose y -> yT, block-diag-G matmul -> featsT, 16 accumulating
       matmuls vs the reordered W^T shard.
  ACT: all PSUM -> SBUF copies (keeps DVE free for the adds).
"""

import numpy as np

import concourse.bacc as bacc
import concourse.mybir as mybir
from concourse.bass_utils import run_bass_kernel_spmd
from concourse.tile import TileContext

N_CORES = 8
B = 32            # batch
H = 512           # image height
WD = 512          # image width
BS = 8            # dct block size
NF = 4            # kept dct coefficients per block
OUT = 512         # linear output dim
RPC = H // N_CORES          # 64 rows per core
IPC = RPC // BS             # 8 block-rows per core
F32 = mybir.dt.float32
F32R = mybir.dt.float32r   # full-rate fp32 PE path (moving dim >= 256)

N_WT_CHUNKS = 8   # weight streamed in 8 chunks of 2 output-tiles each


def _g_mat():
    m = np.arange(NF)[:, None].astype(np.float64)
    l = np.arange(BS)[None, :].astype(np.float64)
    return (np.cos(2.0 * np.pi * m * l / BS) / 8.0).astype(np.float32)  # [4, 8]


def _consts():
    """[128, 384] = identity | G_lo | G_hi.

    G_*[p=(j16,l8), q=(wc2,j16',m4)] = G[m, l] * (j16 == j16'), 'lo' filling
    q < 64 and 'hi' q >= 64, so two accumulating matmuls (rhs = yT of w-chunk
    2*fc, 2*fc+1) yield a [128, 256] featsT tile without partition offsets.
    """
    g = _g_mat()
    block = np.zeros((128, 64), np.float32)
    for j in range(16):
        block[j * 8:(j + 1) * 8, j * 4:(j + 1) * 4] = g.T  # [l, m]
    c = np.zeros((128, 416), np.float32)
    c[:, :128] = np.eye(128, dtype=np.float32)
    c[:, 128:192] = block   # lo: columns 0..63 of G_lo
    c[:, 320:384] = block   # hi: columns 64..127 of G_hi
    c[:, 384:416] = np.tile(np.eye(32, dtype=np.float32), (4, 1))  # 1_4 (x) I_32
    return c


def _build_bass():
    nc = bacc.Bacc("TRN2", target_bir_lowering=False, debug=False)
    # x host-prepped: [h, p=(b16, i8), f=(wh2, a8, w256)]
    x = nc.dram_tensor("x", [2, 128, 2 * BS * 256], F32, kind="ExternalInput")
    # wt host-prepped: [p=(wc2,j16,m), t'=(fc,i), o]
    wt = nc.dram_tensor("wt", [128, 2 * IPC, OUT], F32, kind="ExternalInput")
    out = nc.dram_tensor("out", [B, OUT], F32, kind="ExternalOutput")
    cst_h = nc.inline_tensor(_consts(), "cst")

    with TileContext(nc) as tc:
        with (
            tc.tile_pool(name="sb", bufs=1) as sb,
            tc.tile_pool(name="ps", bufs=1, space="PSUM") as ps,
        ):
            # ---- DMA program order == HWDGE FIFO order ----
            # SP ring: x chunks (wh-major so w-half 0 completes first), then wt
            xt = [
                sb.tile([128, 2 * BS * 256], F32, tag=f"x{h}", name=f"x{h}")
                for h in range(2)
            ]
            for wh in range(2):
                for h in range(2):
                    nc.sync.dma_start(
                        out=xt[h][:, wh * 2048:(wh + 1) * 2048],
                        in_=x.ap()[h][:, wh * 2048:(wh + 1) * 2048],
                    )
            wts = sb.tile([128, 2 * IPC * OUT], F32, tag="wt")
            wr = wt.ap().rearrange("p t o -> p (t o)")
            wck = 2 * IPC * OUT // N_WT_CHUNKS
            for k in range(N_WT_CHUNKS):
                nc.sync.dma_start(
                    out=wts[:, k * wck:(k + 1) * wck],
                    in_=wr[:, k * wck:(k + 1) * wck],
                )
            # ACT ring: constants (identity + G halves)
            cst = sb.tile([128, 416], F32, tag="cst")
            nc.scalar.dma_start(out=cst[:, :], in_=cst_h.ap())
            ident, glo, ghi = cst[:, 0:128], cst[:, 128:256], cst[:, 256:384]
            ones4 = cst[:, 384:416]

            # ---- stage 1: column sums (DVE), per (h, w-half) chunk ----
            ys = [sb.tile([128, WD], F32, tag=f"y{h}", name=f"y{h}") for h in range(2)]
            for wh in range(2):
                for h in range(2):
                    t, base = xt[h], wh * 2048
                    nc.vector.tensor_add(
                        t[:, base:base + 1024],
                        t[:, base:base + 1024],
                        t[:, base + 1024:base + 2048],
                    )
                    nc.vector.tensor_add(
                        t[:, base:base + 512],
                        t[:, base:base + 512],
                        t[:, base + 512:base + 1024],
                    )
                    nc.vector.tensor_add(
                        ys[h][:, wh * 256:(wh + 1) * 256],
                        t[:, base:base + 256],
                        t[:, base + 256:base + 512],
                    )

            # ---- stage 1.5 + 2 per w-half: transpose then block-diag G ----
            fts = []
            for fc in range(2):          # fc == wh
                yts = []
                for wc2 in range(2):
                    wc = 2 * fc + wc2
                    pyt = ps.tile([128, 256], F32, tag=f"pyt{wc}")
                    for h in range(2):
                        nc.tensor.transpose(
                            pyt[:, h * 128:(h + 1) * 128],
                            ys[h][:, wc * 128:(wc + 1) * 128],
                            ident,
                        )
                    yt = sb.tile([128, 256], F32, tag=f"yt{wc}")
                    nc.vector.tensor_copy(yt[:, :], pyt[:, :])
                    yts.append(yt)
                pft = ps.tile([128, 256], F32, tag=f"pft{fc}")
                nc.tensor.matmul(pft[:, :], glo, yts[0][:, :], start=True, stop=False)
                nc.tensor.matmul(pft[:, :], ghi, yts[1][:, :], start=False, stop=True)
                ft = sb.tile([128, 256], F32, tag=f"ft{fc}")
                nc.vector.tensor_copy(ft[:, :], pft[:, :])
                fts.append(ft)

            # ---- stage 3: 16 accumulating matmuls spread over the 4 PE
            # column groups (out partition offset 32*g -> tile_position), so
            # weight loads of one group overlap matmuls of another ----
            pout = ps.tile([128, OUT], F32, tag="pout")
            for fc in range(2):
                for i in range(IPC):
                    t = fc * IPC + i
                    g = t % 4
                    nc.tensor.matmul(
                        pout[32 * g:32 * (g + 1), :],
                        fts[fc][:, i::IPC],
                        wts[:, t * OUT:(t + 1) * OUT],
                        start=(t < 4),
                        stop=(t >= 2 * IPC - 4),
                        tile_position=(0, 32 * g),
                        skip_group_check=True,
                    )
            # sum the 4 groups on-chip: out2[b,o] = sum_g psb[(g,b),o]
            psb = sb.tile([128, OUT], F32, tag="psb")
            nc.vector.tensor_copy(psb[:, :], pout[:, :])
            pred = ps.tile([B, OUT], F32, tag="pred")
            nc.tensor.matmul(pred[:, :], ones4[:, :], psb[:, :], start=True, stop=True)
            outs = sb.tile([B, OUT], F32, tag="outs")
            nc.vector.tensor_copy(outs[:, :], pred[:, :])
            nc.scalar.dma_start(out=out.ap(), in_=outs[:, :])

    nc.compile()
    return nc


_NC_CACHE = None


def _get_nc():
    global _NC_CACHE
    if _NC_CACHE is None:
        _NC_CACHE = _build_bass()
    return _NC_CACHE


def make_in_maps(imgs, weight):
    """Per-core input dicts: shuffled channel-0 row slice + weight shard."""
    wr = weight.reshape(OUT, H // BS, WD // BS, NF)  # [o, i_glob, j, m]
    in_maps = []
    for c in range(N_CORES):
        xc = imgs[:, 0, RPC * c:RPC * (c + 1), :]    # [32, 64, 512]
        # -> [h, (b16, i8), (wh2, a8, w256)]
        xd = xc.reshape(2, 16, IPC, BS, 2, 256).transpose(0, 1, 2, 4, 3, 5)
        xd = np.ascontiguousarray(xd.reshape(2, 128, 2 * BS * 256))
        wc = wr[:, IPC * c:IPC * (c + 1)]            # [o, i, j, m]
        # p = wc2*64 + j16*4 + m (j = fc*32 + wc2*16 + j16), t' = fc*8 + i
        wtc = wc.reshape(OUT, IPC, 2, 2, 16, NF)     # o, i, fc, wc2, j16, m
        wtc = wtc.transpose(3, 4, 5, 2, 1, 0)        # wc2, j16, m, fc, i, o
        wtc = np.ascontiguousarray(wtc.reshape(128, 2 * IPC, OUT))
        in_maps.append({"x": xd, "wt": wtc})
    return in_maps


def kernel(imgs_tensors, weight, bias, block_size=8, num_features=4, **_):
    assert int(block_size) == BS and int(num_features) == NF
    imgs = np.ascontiguousarray(np.asarray(imgs_tensors, dtype=np.float32))
    w = np.ascontiguousarray(np.asarray(weight, dtype=np.float32))
    b = np.asarray(bias, dtype=np.float32)
    assert imgs.shape == (B, 3, H, WD) and w.shape == (OUT, H // BS * WD // BS * NF)

    nc = _get_nc()
    res = run_bass_kernel_spmd(nc, make_in_maps(imgs, w), core_ids=list(range(N_CORES)))
    acc = np.zeros((B, OUT), np.float32)
    for r in res.results:
        acc += r["out"]
    return (acc + b[None, :]).astype(np.float32)


# revision 17
# speedup vs baseline: 1.0037x; 1.0024x over previous
"""DCT-feature-extractor kernel for 8 Trainium2 NeuronCores.

Math collapse: the reference keeps only dct[0, 0:4] of each 8x8 block's 2-D
orthonormal-DFT real part.  Row 0 of the DFT matrix is constant (Fr[0,:] =
1/sqrt(8), Fi[0,:] = 0), so

    feat[m] = sum_l G[m, l] * colsum[l],   G[m, l] = cos(2*pi*m*l/8) / 8,

where colsum[l] is the column sum of the 8x8 block.  The whole module is then

    out[b, o] = sum_{i,j,m} W[o, (i*64+j)*4+m] * feat[b,i,j,m] + bias[o].

Sharding: split the 512 image rows (block-row groups i) and the matching
weight columns across 8 cores -> each core reads 4 MB of image + 4 MB of
weight (the 64 MB of essential HBM traffic / 8, no replication) and emits a
[32, 512] partial product; the host sums partials and adds the bias.

Per-core schedule (HWDGE rings are FIFO per issuing engine, so program order
on nc.sync is the transfer order):
  SP ring: x in 4 x 1MB chunks (host pre-shuffled so each chunk is 8KB-run
           contiguous), then the weight in 8 x 0.5MB chunks so the final
           matmuls can chase arriving chunks.
  ACT ring: constants in, [32, 512] partial out.
  DVE: column-sum tree adds per x chunk.
  PE:  transpose y -> yT, block-diag-G matmul -> featsT, 16 accumulating
       matmuls vs the reordered W^T shard.
  ACT: all PSUM -> SBUF copies (keeps DVE free for the adds).
"""

import numpy as np

import concourse.bacc as bacc
import concourse.mybir as mybir
from concourse.bass_utils import run_bass_kernel_spmd
from concourse.tile import TileContext

N_CORES = 8
B = 32            # batch
H = 512           # image height
WD = 512          # image width
BS = 8            # dct block size
NF = 4            # kept dct coefficients per block
OUT = 512         # linear output dim
RPC = H // N_CORES          # 64 rows per core
IPC = RPC // BS             # 8 block-rows per core
F32 = mybir.dt.float32
F32R = mybir.dt.float32r   # full-rate fp32 PE path (moving dim >= 256)

N_WT_CHUNKS = 8   # weight streamed in 8 chunks of 2 output-tiles each


def _g_mat():
    m = np.arange(NF)[:, None].astype(np.float64)
    l = np.arange(BS)[None, :].astype(np.float64)
    return (np.cos(2.0 * np.pi * m * l / BS) / 8.0).astype(np.float32)  # [4, 8]


def _consts():
    """[128, 384] = identity | G_lo | G_hi.

    G_*[p=(j16,l8), q=(wc2,j16',m4)] = G[m, l] * (j16 == j16'), 'lo' filling
    q < 64 and 'hi' q >= 64, so two accumulating matmuls (rhs = yT of w-chunk
    2*fc, 2*fc+1) yield a [128, 256] featsT tile without partition offsets.
    """
    g = _g_mat()
    block = np.zeros((128, 64), np.float32)
    for j in range(16):
        block[j * 8:(j + 1) * 8, j * 4:(j + 1) * 4] = g.T  # [l, m]
    c = np.zeros((128, 416), np.float32)
    c[:, :128] = np.eye(128, dtype=np.float32)
    c[:, 128:192] = block   # lo: columns 0..63 of G_lo
    c[:, 320:384] = block   # hi: columns 64..127 of G_hi
    c[:, 384:416] = np.tile(np.eye(32, dtype=np.float32), (4, 1))  # 1_4 (x) I_32
    return c


def _build_bass():
    nc = bacc.Bacc("TRN2", target_bir_lowering=False, debug=False)
    # x host-prepped: [h, p=(b16, i8), f=(wh2, a8, w256)]
    x = nc.dram_tensor("x", [2, 128, 2 * BS * 256], F32, kind="ExternalInput")
    # wt host-prepped: [p, 416 consts | t'=(fc,i) x o]
    wt = nc.dram_tensor("wt", [128, 416 + 2 * IPC * OUT], F32, kind="ExternalInput")
    out = nc.dram_tensor("out", [B, OUT], F32, kind="ExternalOutput")

    with TileContext(nc) as tc:
        with (
            tc.tile_pool(name="sb", bufs=1) as sb,
            tc.tile_pool(name="ps", bufs=1, space="PSUM") as ps,
        ):
            # ---- DMA program order == HWDGE FIFO order on the SP ring ----
            # consts first (tiny), then x chunks (wh-major so w-half 0
            # completes first), then the weight stream the final matmuls chase
            wts = sb.tile([128, 416 + 2 * IPC * OUT], F32, tag="wt")
            nc.sync.dma_start(out=wts[:, 0:416], in_=wt.ap()[:, 0:416])
            ident, glo, ghi = wts[:, 0:128], wts[:, 128:256], wts[:, 256:384]
            ones4 = wts[:, 384:416]
            xt = [
                sb.tile([128, 2 * BS * 256], F32, tag=f"x{h}", name=f"x{h}")
                for h in range(2)
            ]
            for wh in range(2):
                for h in range(2):
                    nc.sync.dma_start(
                        out=xt[h][:, wh * 2048:(wh + 1) * 2048],
                        in_=x.ap()[h][:, wh * 2048:(wh + 1) * 2048],
                    )
            wck = 2 * IPC * OUT // N_WT_CHUNKS
            for k in range(N_WT_CHUNKS):
                nc.sync.dma_start(
                    out=wts[:, 416 + k * wck:416 + (k + 1) * wck],
                    in_=wt.ap()[:, 416 + k * wck:416 + (k + 1) * wck],
                )

            # ---- stage 1: column sums (DVE), per (h, w-half) chunk ----
            ys = [sb.tile([128, WD], F32, tag=f"y{h}", name=f"y{h}") for h in range(2)]
            for wh in range(2):
                for h in range(2):
                    t, base = xt[h], wh * 2048
                    nc.vector.tensor_add(
                        t[:, base:base + 1024],
                        t[:, base:base + 1024],
                        t[:, base + 1024:base + 2048],
                    )
                    nc.vector.tensor_add(
                        t[:, base:base + 512],
                        t[:, base:base + 512],
                        t[:, base + 512:base + 1024],
                    )
                    nc.vector.tensor_add(
                        ys[h][:, wh * 256:(wh + 1) * 256],
                        t[:, base:base + 256],
                        t[:, base + 256:base + 512],
                    )

            # ---- stage 1.5 + 2 per w-half: transpose then block-diag G ----
            fts = []
            for fc in range(2):          # fc == wh
                yts = []
                for wc2 in range(2):
                    wc = 2 * fc + wc2
                    pyt = ps.tile([128, 256], F32, tag=f"pyt{wc}")
                    for h in range(2):
                        nc.tensor.transpose(
                            pyt[:, h * 128:(h + 1) * 128],
                            ys[h][:, wc * 128:(wc + 1) * 128],
                            ident,
                        )
                    yt = sb.tile([128, 256], F32, tag=f"yt{wc}")
                    nc.vector.tensor_copy(yt[:, :], pyt[:, :])
                    yts.append(yt)
                pft = ps.tile([128, 256], F32, tag=f"pft{fc}")
                nc.tensor.matmul(pft[:, :], glo, yts[0][:, :], start=True, stop=False)
                nc.tensor.matmul(pft[:, :], ghi, yts[1][:, :], start=False, stop=True)
                ft = sb.tile([128, 256], F32, tag=f"ft{fc}")
                nc.vector.tensor_copy(ft[:, :], pft[:, :])
                fts.append(ft)

            # ---- stage 3: 16 accumulating matmuls spread over the 4 PE
            # column groups (out partition offset 32*g -> tile_position), so
            # weight loads of one group overlap matmuls of another ----
            pout = ps.tile([128, OUT], F32, tag="pout")
            for fc in range(2):
                for i in range(IPC):
                    t = fc * IPC + i
                    g = t % 4
                    nc.tensor.matmul(
                        pout[32 * g:32 * (g + 1), :],
                        fts[fc][:, i::IPC],
                        wts[:, 416 + t * OUT:416 + (t + 1) * OUT],
                        start=(t < 4),
                        stop=(t >= 2 * IPC - 4),
                        tile_position=(0, 32 * g),
                        skip_group_check=True,
                    )
            # sum the 4 groups on-chip: out2[b,o] = sum_g psb[(g,b),o]
            psb = sb.tile([128, OUT], F32, tag="psb")
            nc.vector.tensor_copy(psb[:, :], pout[:, :])
            pred = ps.tile([B, OUT], F32, tag="pred")
            nc.tensor.matmul(pred[:, :], ones4[:, :], psb[:, :], start=True, stop=True)
            outs = sb.tile([B, OUT], F32, tag="outs")
            nc.vector.tensor_copy(outs[:, :], pred[:, :])
            nc.scalar.dma_start(out=out.ap(), in_=outs[:, :])

    nc.compile()
    return nc


_NC_CACHE = None


def _get_nc():
    global _NC_CACHE
    if _NC_CACHE is None:
        _NC_CACHE = _build_bass()
    return _NC_CACHE


_CST = _consts()


def make_in_maps(imgs, weight):
    """Per-core input dicts: shuffled channel-0 row slice + weight shard."""
    wr = weight.reshape(OUT, H // BS, WD // BS, NF)  # [o, i_glob, j, m]
    in_maps = []
    for c in range(N_CORES):
        xc = imgs[:, 0, RPC * c:RPC * (c + 1), :]    # [32, 64, 512]
        # -> [h, (b16, i8), (wh2, a8, w256)]
        xd = xc.reshape(2, 16, IPC, BS, 2, 256).transpose(0, 1, 2, 4, 3, 5)
        xd = np.ascontiguousarray(xd.reshape(2, 128, 2 * BS * 256))
        wc = wr[:, IPC * c:IPC * (c + 1)]            # [o, i, j, m]
        # p = wc2*64 + j16*4 + m (j = fc*32 + wc2*16 + j16), t' = fc*8 + i
        wtc = wc.reshape(OUT, IPC, 2, 2, 16, NF)     # o, i, fc, wc2, j16, m
        wtc = wtc.transpose(3, 4, 5, 2, 1, 0)        # wc2, j16, m, fc, i, o
        wtc = np.concatenate([_CST, wtc.reshape(128, 2 * IPC * OUT)], axis=1)
        in_maps.append({"x": xd, "wt": np.ascontiguousarray(wtc)})
    return in_maps


def kernel(imgs_tensors, weight, bias, block_size=8, num_features=4, **_):
    assert int(block_size) == BS and int(num_features) == NF
    imgs = np.ascontiguousarray(np.asarray(imgs_tensors, dtype=np.float32))
    w = np.ascontiguousarray(np.asarray(weight, dtype=np.float32))
    b = np.asarray(bias, dtype=np.float32)
    assert imgs.shape == (B, 3, H, WD) and w.shape == (OUT, H // BS * WD // BS * NF)

    nc = _get_nc()
    res = run_bass_kernel_spmd(nc, make_in_maps(imgs, w), core_ids=list(range(N_CORES)))
    acc = np.zeros((B, OUT), np.float32)
    for r in res.results:
        acc += r["out"]
    return (acc + b[None, :]).astype(np.float32)


# revision 18
# speedup vs baseline: 1.0771x; 1.0731x over previous
"""DCT-feature-extractor kernel for 8 Trainium2 NeuronCores.

Math collapse: the reference keeps only dct[0, 0:4] of each 8x8 block's 2-D
orthonormal-DFT real part.  Row 0 of the DFT matrix is constant (Fr[0,:] =
1/sqrt(8), Fi[0,:] = 0), so

    feat[m] = sum_l G[m, l] * colsum[l],   G[m, l] = cos(2*pi*m*l/8) / 8,

where colsum[l] is the column sum of the 8x8 block.  The whole module is then

    out[b, o] = sum_{i,j,m} W[o, (i*64+j)*4+m] * feat[b,i,j,m] + bias[o].

Sharding: split the 512 image rows (block-row groups i) and the matching
weight columns across 8 cores -> each core reads 4 MB of image + 4 MB of
weight (the 64 MB of essential HBM traffic / 8, no replication) and emits a
[32, 512] partial product; the host sums partials and adds the bias.

Per-core schedule (HWDGE rings are FIFO per issuing engine, so program order
on nc.sync is the transfer order):
  SP ring: x in 4 x 1MB chunks (host pre-shuffled so each chunk is 8KB-run
           contiguous), then the weight in 8 x 0.5MB chunks so the final
           matmuls can chase arriving chunks.
  ACT ring: constants in, [32, 512] partial out.
  DVE: column-sum tree adds per x chunk.
  PE:  transpose y -> yT, block-diag-G matmul -> featsT, 16 accumulating
       matmuls vs the reordered W^T shard.
  ACT: all PSUM -> SBUF copies (keeps DVE free for the adds).
"""

import numpy as np

import concourse.bacc as bacc
import concourse.mybir as mybir
from concourse.bass_utils import run_bass_kernel_spmd
from concourse.tile import TileContext

N_CORES = 8
B = 32            # batch
H = 512           # image height
WD = 512          # image width
BS = 8            # dct block size
NF = 4            # kept dct coefficients per block
OUT = 512         # linear output dim
RPC = H // N_CORES          # 64 rows per core
IPC = RPC // BS             # 8 block-rows per core
F32 = mybir.dt.float32
F32R = mybir.dt.float32r   # full-rate fp32 PE path (moving dim >= 256)

N_WT_CHUNKS = 8   # weight streamed in 8 chunks of 2 output-tiles each


def _g_mat():
    m = np.arange(NF)[:, None].astype(np.float64)
    l = np.arange(BS)[None, :].astype(np.float64)
    return (np.cos(2.0 * np.pi * m * l / BS) / 8.0).astype(np.float32)  # [4, 8]


def _consts():
    """[128, 384] = identity | G_lo | G_hi.

    G_*[p=(j16,l8), q=(wc2,j16',m4)] = G[m, l] * (j16 == j16'), 'lo' filling
    q < 64 and 'hi' q >= 64, so two accumulating matmuls (rhs = yT of w-chunk
    2*fc, 2*fc+1) yield a [128, 256] featsT tile without partition offsets.
    """
    g = _g_mat()
    block = np.zeros((128, 64), np.float32)
    for j in range(16):
        block[j * 8:(j + 1) * 8, j * 4:(j + 1) * 4] = g.T  # [l, m]
    c = np.zeros((128, 416), np.float32)
    c[:, :128] = np.eye(128, dtype=np.float32)
    c[:, 128:192] = block   # lo: columns 0..63 of G_lo
    c[:, 320:384] = block   # hi: columns 64..127 of G_hi
    c[:, 384:416] = np.tile(np.eye(32, dtype=np.float32), (4, 1))  # 1_4 (x) I_32
    return c


def _build_bass():
    nc = bacc.Bacc("TRN2", target_bir_lowering=False, debug=False)
    # x host-prepped: [h, p=(b16, i8), f=(wh2, a8, w256)]
    x = nc.dram_tensor("x", [2, 128, 2 * BS * 256], F32, kind="ExternalInput")
    # wt host-prepped: [p, 416 consts | t'=(fc,i) x o]
    wt = nc.dram_tensor("wt", [128, 416 + 2 * IPC * OUT], F32, kind="ExternalInput")
    out = nc.dram_tensor("out", [B, OUT], F32, kind="ExternalOutput")

    with TileContext(nc) as tc:
        with (
            tc.tile_pool(name="sb", bufs=1) as sb,
            tc.tile_pool(name="ps", bufs=1, space="PSUM") as ps,
        ):
            # ---- DMA program order == HWDGE FIFO order on the SP ring ----
            # consts first (tiny), then x chunks (wh-major so w-half 0
            # completes first), then the weight stream the final matmuls chase
            wts = sb.tile([128, 416 + 2 * IPC * OUT], F32, tag="wt")
            nc.scalar.dma_start(out=wts[:, 0:416], in_=wt.ap()[:, 0:416])
            ident, glo, ghi = wts[:, 0:128], wts[:, 128:256], wts[:, 256:384]
            ones4 = wts[:, 384:416]
            xt = [
                sb.tile([128, 2 * BS * 256], F32, tag=f"x{h}", name=f"x{h}")
                for h in range(2)
            ]
            for wh in range(2):
                for h in range(2):
                    nc.sync.dma_start(
                        out=xt[h][:, wh * 2048:(wh + 1) * 2048],
                        in_=x.ap()[h][:, wh * 2048:(wh + 1) * 2048],
                    )
            wck = 2 * IPC * OUT // N_WT_CHUNKS
            for k in range(N_WT_CHUNKS):
                nc.sync.dma_start(
                    out=wts[:, 416 + k * wck:416 + (k + 1) * wck],
                    in_=wt.ap()[:, 416 + k * wck:416 + (k + 1) * wck],
                )

            # ---- stage 1: column sums (DVE), per (h, w-half) chunk ----
            ys = [sb.tile([128, WD], F32, tag=f"y{h}", name=f"y{h}") for h in range(2)]
            for wh in range(2):
                for h in range(2):
                    t, base = xt[h], wh * 2048
                    nc.vector.tensor_add(
                        t[:, base:base + 1024],
                        t[:, base:base + 1024],
                        t[:, base + 1024:base + 2048],
                    )
                    nc.vector.tensor_add(
                        t[:, base:base + 512],
                        t[:, base:base + 512],
                        t[:, base + 512:base + 1024],
                    )
                    nc.vector.tensor_add(
                        ys[h][:, wh * 256:(wh + 1) * 256],
                        t[:, base:base + 256],
                        t[:, base + 256:base + 512],
                    )

            # ---- stage 1.5 + 2 per w-half: transpose then block-diag G ----
            fts = []
            for fc in range(2):          # fc == wh
                yts = []
                for wc2 in range(2):
                    wc = 2 * fc + wc2
                    pyt = ps.tile([128, 256], F32, tag=f"pyt{wc}")
                    for h in range(2):
                        nc.tensor.transpose(
                            pyt[:, h * 128:(h + 1) * 128],
                            ys[h][:, wc * 128:(wc + 1) * 128],
                            ident,
                        )
                    yt = sb.tile([128, 256], F32, tag=f"yt{wc}")
                    nc.vector.tensor_copy(yt[:, :], pyt[:, :])
                    yts.append(yt)
                pft = ps.tile([128, 256], F32, tag=f"pft{fc}")
                nc.tensor.matmul(pft[:, :], glo, yts[0][:, :], start=True, stop=False)
                nc.tensor.matmul(pft[:, :], ghi, yts[1][:, :], start=False, stop=True)
                ft = sb.tile([128, 256], F32, tag=f"ft{fc}")
                nc.vector.tensor_copy(ft[:, :], pft[:, :])
                fts.append(ft)

            # ---- stage 3: 16 accumulating matmuls spread over the 4 PE
            # column groups (out partition offset 32*g -> tile_position), so
            # weight loads of one group overlap matmuls of another ----
            pout = ps.tile([128, OUT], F32, tag="pout")
            for fc in range(2):
                for i in range(IPC):
                    t = fc * IPC + i
                    g = t % 4
                    nc.tensor.matmul(
                        pout[32 * g:32 * (g + 1), :],
                        fts[fc][:, i::IPC],
                        wts[:, 416 + t * OUT:416 + (t + 1) * OUT],
                        start=(t < 4),
                        stop=(t >= 2 * IPC - 4),
                        tile_position=(0, 32 * g),
                        skip_group_check=True,
                    )
            # sum the 4 groups on-chip: out2[b,o] = sum_g psb[(g,b),o]
            psb = sb.tile([128, OUT], F32, tag="psb")
            nc.vector.tensor_copy(psb[:, :], pout[:, :])
            pred = ps.tile([B, OUT], F32, tag="pred")
            nc.tensor.matmul(pred[:, :], ones4[:, :], psb[:, :], start=True, stop=True)
            outs = sb.tile([B, OUT], F32, tag="outs")
            nc.vector.tensor_copy(outs[:, :], pred[:, :])
            nc.scalar.dma_start(out=out.ap(), in_=outs[:, :])

    nc.compile()
    return nc


_NC_CACHE = None


def _get_nc():
    global _NC_CACHE
    if _NC_CACHE is None:
        _NC_CACHE = _build_bass()
    return _NC_CACHE


_CST = _consts()


def make_in_maps(imgs, weight):
    """Per-core input dicts: shuffled channel-0 row slice + weight shard."""
    wr = weight.reshape(OUT, H // BS, WD // BS, NF)  # [o, i_glob, j, m]
    in_maps = []
    for c in range(N_CORES):
        xc = imgs[:, 0, RPC * c:RPC * (c + 1), :]    # [32, 64, 512]
        # -> [h, (b16, i8), (wh2, a8, w256)]
        xd = xc.reshape(2, 16, IPC, BS, 2, 256).transpose(0, 1, 2, 4, 3, 5)
        xd = np.ascontiguousarray(xd.reshape(2, 128, 2 * BS * 256))
        wc = wr[:, IPC * c:IPC * (c + 1)]            # [o, i, j, m]
        # p = wc2*64 + j16*4 + m (j = fc*32 + wc2*16 + j16), t' = fc*8 + i
        wtc = wc.reshape(OUT, IPC, 2, 2, 16, NF)     # o, i, fc, wc2, j16, m
        wtc = wtc.transpose(3, 4, 5, 2, 1, 0)        # wc2, j16, m, fc, i, o
        wtc = np.concatenate([_CST, wtc.reshape(128, 2 * IPC * OUT)], axis=1)
        in_maps.append({"x": xd, "wt": np.ascontiguousarray(wtc)})
    return in_maps


def kernel(imgs_tensors, weight, bias, block_size=8, num_features=4, **_):
    assert int(block_size) == BS and int(num_features) == NF
    imgs = np.ascontiguousarray(np.asarray(imgs_tensors, dtype=np.float32))
    w = np.ascontiguousarray(np.asarray(weight, dtype=np.float32))
    b = np.asarray(bias, dtype=np.float32)
    assert imgs.shape == (B, 3, H, WD) and w.shape == (OUT, H // BS * WD // BS * NF)

    nc = _get_nc()
    res = run_bass_kernel_spmd(nc, make_in_maps(imgs, w), core_ids=list(range(N_CORES)))
    acc = np.zeros((B, OUT), np.float32)
    for r in res.results:
        acc += r["out"]
    return (acc + b[None, :]).astype(np.float32)
